# revision 1
# baseline (speedup 1.0000x reference)
"""Trainium2 Bass kernel for nn_CenterAttention.

Math (per batch b):
  spatial: center-query MHA over n=256 patches -> spatial[b, 1024]
  spectral: attention over feature dim: L = q_s @ k_s.T * scale,
            W = softmax(L, axis=-1); out[b, n, :] = spatial[b] @ W[b]
The output rows are identical across n (spatial_out is a broadcast), so the
device computes out_row[b, e] = sum_i (spatial[b,i]/S[b,i]) * exp(scale*L[b,i,e])
and the host broadcasts to [64, 256, 1024].

Sharding: pure data-parallel over batch, 8 batches per core, weights
replicated. All matmul operands use float32r (fp32 storage,
~1e-3 matmul precision at 1 cyc/row on the PE for N>=256).
"""

import sys

sys.path.insert(0, "/opt/trn_rl_repo")

import numpy as np

import concourse.bass as bass
import concourse.tile as tile
from concourse import bacc, mybir
from concourse.bass_utils import run_bass_kernel_spmd
from concourse.masks import make_identity

F32 = mybir.dt.float32
F32R = mybir.dt.float32r
F16 = mybir.dt.float16
EXP_SHIFT = 4.0   # exp(scale*L - C): keeps fp16 E below overflow; C cancels in w

N_CORES = 8
B = 64
PER = B // N_CORES          # 8 batches per core
N = 256                      # patches
D = 1024                     # dim
HEADS = 8
DH = 64
INNER = HEADS * DH           # 512
SCALE = DH ** -0.5           # 0.125

_CACHE = {}


def _build():
    nc = bacc.Bacc("TRN2", target_bir_lowering=False, debug=False,
                   num_devices=N_CORES)

    # ---- DRAM I/O (per-core shapes; host pre-packs to [128, X] tiles) ----
    d_x = nc.dram_tensor("x", [PER, 128, 2 * D], F32R, kind="ExternalInput").ap()
    d_xt = nc.dram_tensor("xt", [PER, 128, 8 * N], F32R, kind="ExternalInput").ap()
    d_xct = nc.dram_tensor("xct", [128, 8 * PER], F32R, kind="ExternalInput").ap()
    d_wq = nc.dram_tensor("wq", [128, 8 * INNER], F32R, kind="ExternalInput").ap()
    d_wkt = nc.dram_tensor("wkt", [128, 4 * D], F32R, kind="ExternalInput").ap()
    d_wv = nc.dram_tensor("wv", [128, 8 * INNER], F32R, kind="ExternalInput").ap()
    d_wout = nc.dram_tensor("wout", [128, 4 * D], F32R, kind="ExternalInput").ap()
    d_wspec = nc.dram_tensor("wspec", [128, 2 * INNER], F32R, kind="ExternalInput").ap()
    d_bout = nc.dram_tensor("bout", [128, 8], F32, kind="ExternalInput").ap()
    d_out = nc.dram_tensor("out", [PER, D], F32, kind="ExternalOutput").ap()

    with tile.TileContext(nc) as tc:
        _emit(nc, tc, d_x, d_xt, d_xct, d_wq, d_wkt, d_wv, d_wout, d_wspec,
              d_bout, d_out)
    nc.compile()
    return nc


def _emit(nc, tc, d_x, d_xt, d_xct, d_wq, d_wkt, d_wv, d_wout, d_wspec,
          d_bout, d_out):
    import contextlib
    ctx = contextlib.ExitStack()
    with ctx:
        const = ctx.enter_context(tc.tile_pool(name="const", bufs=1))
        sb = ctx.enter_context(tc.tile_pool(name="sb", bufs=2))
        sb3 = ctx.enter_context(tc.tile_pool(name="sb3", bufs=3))
        pbig = ctx.enter_context(tc.tile_pool(name="pbig", bufs=2, space="PSUM"))
        pmid = ctx.enter_context(tc.tile_pool(name="pmid", bufs=3, space="PSUM"))
        pout = ctx.enter_context(tc.tile_pool(name="pout", bufs=1, space="PSUM"))

        # ---- load constants ----
        def cload(dram, shape, tag, dt=F32R, chunks=1):
            t = const.tile(shape, dt, tag=tag)
            w = shape[1] // chunks
            for c in range(chunks):
                nc.sync.dma_start(t[:, c * w:(c + 1) * w],
                                  dram[:, c * w:(c + 1) * w])
            return t

        def cload_split(dram, width, tag, nt):
            '''Split a [128, width] dram tensor into nt separate tiles so
            consumers only wait on the chunk they read.'''
            w = width // nt
            ts = []
            for c in range(nt):
                t = const.tile([128, w], F32R, tag=f"{tag}{c}", name=f"{tag}{c}")
                nc.sync.dma_start(t[:], dram[:, c * w:(c + 1) * w])
                ts.append(t)
            return ts

        # phase-0-critical constants first (SP DMA ring is FIFO).
        # wq chunk t serves qT t-block t; wkt chunk c serves qWT ib in
        # [4c, 4c+4) (host packs them that way).
        xct = cload(d_xct, [128, 8 * PER], "xct")

        # batch-input prefetch helpers
        xt_tiles, x_tiles = {}, {}

        def load_xt(b):
            t = sb3.tile([128, 8 * N], F32R, tag="xtb", name=f"xt_sb{b}")
            nc.sync.dma_start(t[:], d_xt[b])
            xt_tiles[b] = [t[:, 0:4 * N], t[:, 4 * N:8 * N]]

        def load_x(b):
            t = sb.tile([128, 2 * D], F32R, tag="xb", name=f"x_sb{b}",
                        bufs=3)
            nc.sync.dma_start(t[:], d_x[b])
            x_tiles[b] = t

        def xsl(b, kt, it):
            t = x_tiles[b]
            if isinstance(t, list):
                return t[kt][:, it * 512:(it + 1) * 512]
            return t[:, kt * D + it * 512: kt * D + (it + 1) * 512]

        x0_ts, wspec_t = [], []
        for kt in range(2):
            xt0k = sb.tile([128, D], F32R, tag=f"x0k{kt}", name=f"x0k{kt}",
                           bufs=1)
            nc.sync.dma_start(xt0k[:], d_x[0][:, kt * D:(kt + 1) * D])
            x0_ts.append(xt0k)
            wsp = const.tile([128, INNER], F32R, tag=f"wspec{kt}",
                             name=f"wspec{kt}")
            nc.sync.dma_start(wsp[:], d_wspec[:, kt * INNER:(kt + 1) * INNER])
            wspec_t.append(wsp)
        x_tiles[0] = x0_ts
        wq_t = cload_split(d_wq, 8 * INNER, "wq", 4)
        load_x(1)
        load_xt(0)
        wkt_t = cload_split(d_wkt, 4 * D, "wkt", 2)
        wv_t = cload_split(d_wv, 8 * INNER, "wv", 2)
        load_xt(1)
        load_xt(2)
        wout = cload(d_wout, [128, 4 * D], "wout")
        boutT = cload(d_bout, [128, 8], "bout", dt=F32)
        ident = const.tile([128, 128], F32, tag="ident")
        make_identity(nc, ident[:])
        # fp32 zero scratch (DVE memset cannot write f32r; copy rounds instead)
        zeros1 = const.tile([128, 1], F32, tag="zeros1")
        nc.vector.memset(zeros1[:], 0.0)
        neg_shift = const.tile([128, 1], F32, tag="neg_shift")
        nc.vector.memset(neg_shift[:], -EXP_SHIFT)

        # ---- phase 0: qT [512, PER] = Wq.T @ xcT  (per dhg-block of 128) ----
        # qT[t*128+p, b], t in 0..3
        qT = const.tile([128, 4 * PER], F32R, tag="qT")
        for t in range(4):
            ps = pmid.tile([128, PER], F32, tag="mid")
            for k in range(8):
                nc.tensor.matmul(
                    ps[:], wq_t[t][:, k * 128:(k + 1) * 128],
                    xct[:, k * PER:(k + 1) * PER],
                    start=(k == 0), stop=(k == 7))
            nc.vector.tensor_copy(qT[:, t * PER:(t + 1) * PER], ps[:])

        # ---- Q_blk [512, 8*PER] block-diagonal: col b*8+h = qT[:, b] on head h
        qblk = const.tile([128, 4 * 8 * PER], F32R, tag="qblk")
        nc.vector.tensor_copy(qblk[:], zeros1[:].to_broadcast((128, 4 * 8 * PER)))
        for t in range(4):
            for b in range(PER):
                # partitions 0:64 hold head 2t, 64:128 hold head 2t+1
                c0 = t * 64 + b * 8
                nc.vector.tensor_copy(
                    qblk[0:64, c0 + 2 * t: c0 + 2 * t + 1],
                    qT[0:64, t * PER + b: t * PER + b + 1])
                nc.vector.tensor_copy(
                    qblk[64:128, c0 + 2 * t + 1: c0 + 2 * t + 2],
                    qT[64:128, t * PER + b: t * PER + b + 1])

        # ---- qWT [1024, 8*PER]: qWT[d, b*8+h] = sum_hdh WkT[hdh, d] Qblk[hdh, bh]
        qwt = const.tile([128, 8 * 8 * PER], F32R, tag="qwt")
        for ib in range(8):
            ps = pmid.tile([128, 8 * PER], F32, tag="mid")
            for kt in range(4):
                nc.tensor.matmul(
                    ps[:],
                    wkt_t[ib // 4][:, kt * 512 + (ib % 4) * 128:
                                   kt * 512 + (ib % 4 + 1) * 128],
                    qblk[:, kt * 64: (kt + 1) * 64],
                    start=(kt == 0), stop=(kt == 3))
            nc.vector.tensor_copy(qwt[:, ib * 64:(ib + 1) * 64], ps[:])

        # persistent output accumulator in PSUM (1 bank). Row 32*g + b holds
        # batch b's partial sum for col-group g = 2*(ib%2) + jt; the four
        # col-groups let final matmuls run concurrently in the PE array.
        out_ps = pout.tile([128, 512], F32, tag="out")
        # Zero the accumulator data; all final matmuls then use start=False,
        # which is correct regardless of stale has_written bits (add-to-zero
        # and overwrite give the same result).
        nc.vector.memset(out_ps[:], 0.0)

        # attT_all[ht]: [128, PER] fp32, written column-by-column per batch
        attT_all = [const.tile([128, PER], F32R, tag=f"attT{t}", name=f"attT{t}")
                    for t in range(4)]
        # spatialT per group: spT[ib] [128, 4]
        spT = [None] * 8

        def spatial_chain(b):
            """Spatial branch for batch b -> writes attT_all[:][:, b]."""
            if b not in xt_tiles:
                load_xt(b)
            xth = xt_tiles[b]

            # sp logits [HEADS, N] = qWT[:, b*8:b*8+8].T @ xT_b
            lg = pmid.tile([HEADS, N], F32, tag="mid")
            for k in range(8):
                nc.tensor.matmul(
                    lg[:], qwt[:, k * 64 + b * 8: k * 64 + (b + 1) * 8],
                    xth[k // 4][:, (k % 4) * N:(k % 4 + 1) * N],
                    start=(k == 0), stop=(k == 7))
            # softmax over free dim (no max-subtraction; logits are O(5))
            esp = sb.tile([HEADS, N], F32, tag="esp")
            ssp = sb.tile([HEADS, 1], F32, tag="ssp")
            nc.scalar.activation(esp[:], lg[:], mybir.ActivationFunctionType.Exp,
                                 scale=SCALE, accum_out=ssp[:])
            rsp = sb.tile([HEADS, 1], F32, tag="rsp")
            nc.vector.reciprocal(rsp[:], ssp[:])
            attn = sb.tile([HEADS, N], F32, tag="attn")
            nc.vector.tensor_scalar_mul(attn[:], esp[:], rsp[:])

            # attnT [N, HEADS] via PE transpose (2 blocks of 128)
            espT = sb.tile([128, 2 * HEADS], F32R, tag="espT")
            for nb in range(2):
                ps = pmid.tile([128, HEADS], F32, tag="mid")
                nc.tensor.transpose(ps[:], attn[:, nb * 128:(nb + 1) * 128],
                                    ident[0:HEADS, 0:HEADS])
                nc.vector.tensor_copy(espT[:, nb * HEADS:(nb + 1) * HEADS], ps[:])

            # v [N, INNER]: v[nb*128+p, hdh] ; lhsT = xT_b block, rhs = Wv
            v_sb = sb.tile([128, 2 * INNER], F32R, tag="vb", bufs=3)
            for nb in range(2):
                ps = pmid.tile([128, INNER], F32, tag="mid")
                for k in range(8):
                    nc.tensor.matmul(
                        ps[:],
                        xth[k // 4][:, (k % 4) * N + nb * 128:
                                    (k % 4) * N + (nb + 1) * 128],
                        wv_t[k // 4][:, (k % 4) * INNER:(k % 4 + 1) * INNER],
                        start=(k == 0), stop=(k == 7))
                if nb == 0:
                    nc.scalar.copy(v_sb[:, 0:INNER], ps[:])
                else:
                    nc.vector.tensor_copy(v_sb[:, INNER:2 * INNER], ps[:])

            # attended.T [INNER, 1] for this batch: per hdh-block ht
            for ht in range(4):
                ps = pmid.tile([128, HEADS], F32, tag="mid")
                for nb in range(2):
                    nc.tensor.matmul(
                        ps[:],
                        v_sb[:, nb * INNER + ht * 128: nb * INNER + (ht + 1) * 128],
                        espT[:, nb * HEADS:(nb + 1) * HEADS],
                        start=(nb == 0), stop=(nb == 1))
                # rows 0:64 are head 2ht -> col 2ht; rows 64:128 head 2ht+1
                nc.vector.tensor_copy(attT_all[ht][0:64, b:b + 1],
                                      ps[0:64, 2 * ht: 2 * ht + 1])
                nc.vector.tensor_copy(attT_all[ht][64:128, b:b + 1],
                                      ps[64:128, 2 * ht + 1: 2 * ht + 2])

        def spec_qsks(b):
            """q_sT / k_sT [256, 1024] as 2+2 partition-tiles [128, 1024]."""
            if b not in x_tiles:
                load_x(b)
            qsks = []
            for mb in range(4):      # 0,1 -> q_sT ; 2,3 -> k_sT
                ps = pbig.tile([128, D], F32, tag="big")
                for it in range(2):
                    for kt in range(2):
                        nc.tensor.matmul(
                            ps[:, it * 512:(it + 1) * 512],
                            wspec_t[kt][:, mb * 128:(mb + 1) * 128],
                            xsl(b, kt, it),
                            start=(kt == 0), stop=(kt == 1))
                t = sb.tile([128, D], F32R, tag=f"qsks{mb}", name=f"qsks{mb}_{b}")
                if mb == 3:
                    nc.scalar.copy(t[:], ps[:])   # parallel to DVE's 3rd copy
                else:
                    nc.vector.tensor_copy(t[:], ps[:])
                qsks.append(t)
            return qsks

        def spec_Lexp(b, qsks, ib):
            """Spec logits i-block ib -> exp (fp16 E) + row-sum. The last
            i-block uses pmid half-tiles so its exp latency does not hold the
            pbig slots the next batch's qs/ks matmuls need."""
            e_sb = sb3.tile([128, D], F16, tag="e", name=f"e_{b}_{ib}", bufs=10)
            s_t = sb3.tile([128, 1], F32, tag="s", name=f"s_{b}_{ib}", bufs=10)
            if ib == 7:
                s_h = sb3.tile([128, 1], F32, tag="s_h", name=f"sh_{b}_{ib}")
                for jt in range(2):
                    ps = pmid.tile([128, 512], F32, tag="mid")
                    for kt in range(2):
                        nc.tensor.matmul(
                            ps[:],
                            qsks[kt][:, ib * 128:(ib + 1) * 128],
                            qsks[2 + kt][:, jt * 512:(jt + 1) * 512],
                            start=(kt == 0), stop=(kt == 1))
                    nc.scalar.activation(
                        e_sb[:, jt * 512:(jt + 1) * 512], ps[:],
                        mybir.ActivationFunctionType.Exp,
                        scale=SCALE, bias=neg_shift[:],
                        accum_out=(s_t[:] if jt == 0 else s_h[:]))
                nc.vector.tensor_add(s_t[:], s_t[:], s_h[:])
                return e_sb, s_t
            ps = pbig.tile([128, D], F32, tag="big")
            for jt in range(2):
                for kt in range(2):
                    nc.tensor.matmul(
                        ps[:, jt * 512:(jt + 1) * 512],
                        qsks[kt][:, ib * 128:(ib + 1) * 128],
                        qsks[2 + kt][:, jt * 512:(jt + 1) * 512],
                        start=(kt == 0), stop=(kt == 1))
            nc.scalar.activation(e_sb[:], ps[:],
                                 mybir.ActivationFunctionType.Exp,
                                 scale=SCALE, bias=neg_shift[:],
                                 accum_out=s_t[:])
            return e_sb, s_t

        def spec_final(b, g, ib, e_sb, s_t, last):
            """w8 column + two col-group matmuls into out_ps."""
            rec = sb3.tile([128, 1], F32, tag="rec", name=f"rec_{b}_{ib}")
            nc.vector.reciprocal(rec[:], s_t[:])
            w8 = sb3.tile([128, PER], F16, tag=f"w8_{ib}", name=f"w8_{b}_{ib}")
            nc.vector.tensor_copy(w8[:], zeros1[:].to_broadcast((128, PER)))
            nc.vector.tensor_mul(w8[:, b: b + 1],
                                 spT[ib][:, b - 4 * g: b - 4 * g + 1], rec[:])
            for jt in range(2):
                cg = 32 * (2 * (ib % 2) + jt)
                nc.tensor.matmul(
                    out_ps[cg: cg + PER, :],
                    w8[:], e_sb[:, jt * 512:(jt + 1) * 512],
                    start=False,
                    stop=(last and ib == 7),
                    tile_position=(0, cg),
                    skip_group_check=True)

        qsks_d = {}

        def run_batch(b, g, nxt=None, last=False):
            """Spec batch with the NEXT batch's qs/ks emitted mid-L-loop so
            PE has ready matmuls while this batch's trailing exps drain the
            pbig slots."""
            if b not in qsks_d:
                qsks_d[b] = spec_qsks(b)
            qsks = qsks_d[b]
            for ib in range(6):
                e_sb, s_t = spec_Lexp(b, qsks, ib)
                spec_final(b, g, ib, e_sb, s_t, last)
            if nxt is not None and nxt not in qsks_d:
                qsks_d[nxt] = spec_qsks(nxt)
            for ib in (6, 7):
                e_sb, s_t = spec_Lexp(b, qsks, ib)
                spec_final(b, g, ib, e_sb, s_t, last)

        def spT_phase(g):
            # spatialT for the group: spT[ib] = Wout.T @ attT_cols + bout
            for ib in range(8):
                ps = pmid.tile([128, 4], F32, tag="mid")
                for kt in range(4):
                    nc.tensor.matmul(
                        ps[:], wout[:, kt * D + ib * 128: kt * D + (ib + 1) * 128],
                        attT_all[kt][:, 4 * g: 4 * g + 4],
                        start=(kt == 0), stop=(kt == 3))
                spT[ib] = sb.tile([128, 4], F32, tag=f"spT{ib}", name=f"spT{ib}_{g}")
                nc.vector.tensor_scalar_add(spT[ib][:], ps[:],
                                            boutT[:, ib: ib + 1])

        # ================= main loop =================
        # Batch 0's qs/ks + first L/exp blocks are emitted before spT(0) so
        # that PE has spec work to chew on while the group-0 chains wait on
        # xt DMAs (they only need wspec + x0, which arrive early). The
        # deferred finals only hold the 3 E-pool slots.
        spatial_chain(0)
        qsks_d[0] = spec_qsks(0)
        pre0 = [spec_Lexp(0, qsks_d[0], ib) for ib in range(4)]
        spatial_chain(1)
        qsks_d[1] = spec_qsks(1)
        pre1 = [spec_Lexp(1, qsks_d[1], ib) for ib in range(5)]
        spatial_chain(2)
        spatial_chain(3)
        spT_phase(0)
        for ib in range(4):
            spec_final(0, 0, ib, *pre0[ib], last=False)
        for ib in range(4, 8):
            e_sb, s_t = spec_Lexp(0, qsks_d[0], ib)
            spec_final(0, 0, ib, e_sb, s_t, last=False)
        for ib in range(5):
            spec_final(1, 0, ib, *pre1[ib], last=False)
        for ib in range(5, 6):
            e_sb, s_t = spec_Lexp(1, qsks_d[1], ib)
            spec_final(1, 0, ib, e_sb, s_t, last=False)
        qsks_d[2] = spec_qsks(2)
        for ib in (6, 7):
            e_sb, s_t = spec_Lexp(1, qsks_d[1], ib)
            spec_final(1, 0, ib, e_sb, s_t, last=False)
        run_batch(2, 0, nxt=3)
        run_batch(3, 0, nxt=4)
        spatial_chain(4)
        spatial_chain(5)
        spatial_chain(6)
        spatial_chain(7)
        spT_phase(1)
        run_batch(4, 1, nxt=5)
        run_batch(5, 1, nxt=6)
        qsks_d[7] = spec_qsks(7)
        for ib in range(8):
            e6, s6 = spec_Lexp(6, qsks_d[6], ib)
            spec_final(6, 1, ib, e6, s6, last=False)
            e7, s7 = spec_Lexp(7, qsks_d[7], ib)
            spec_final(7, 1, ib, e7, s7, last=(ib == 7))

        # ---- write out: sum col-group partial rows (groups 0,2 -> j half 0;
        # groups 1,3 -> j half 1). Each DVE op reads at most one PSUM operand.
        t0 = sb.tile([PER, 512], F32, tag="t0", bufs=1)
        t1 = sb.tile([PER, 512], F32, tag="t1", bufs=1)
        out_sb = sb.tile([PER, D], F32, tag="outsb", bufs=1)
        nc.vector.tensor_copy(t0[:], out_ps[0:PER, :])
        nc.vector.tensor_add(out_sb[:, 0:512], t0[:], out_ps[64:64 + PER, :])
        nc.vector.tensor_copy(t1[:], out_ps[32:32 + PER, :])
        nc.vector.tensor_add(out_sb[:, 512:D], t1[:], out_ps[96:96 + PER, :])
        nc.sync.dma_start(d_out[:, :], out_sb[:])


def _prep_inputs(x, Wq, Wkv, Wout, bout, Wspec):
    """Host-side layout prep: slice per core, pack to [128, X] tile layouts."""
    x = np.ascontiguousarray(np.asarray(x, dtype=np.float32))
    Wq = np.asarray(Wq, dtype=np.float32)
    Wkv = np.asarray(Wkv, dtype=np.float32)
    Wout = np.asarray(Wout, dtype=np.float32)
    bout = np.asarray(bout, dtype=np.float32)
    Wspec = np.asarray(Wspec, dtype=np.float32)

    # chunk t of wq_r serves qT t-block t: [p, t, k, jl]
    wq_r = np.ascontiguousarray(
        Wq.reshape(8, 128, 4, 128).transpose(1, 2, 0, 3).reshape(128, 8 * INNER))
    wkt = np.ascontiguousarray(Wkv[:, :INNER].T)       # [512, 1024]
    # chunk c of wkt_r serves qWT i-blocks [4c, 4c+4): [p, c, kt, ibl, jl]
    wkt_r = np.ascontiguousarray(
        wkt.reshape(4, 128, 2, 4, 128).transpose(1, 2, 0, 3, 4).reshape(128, 4 * D))
    wv_r = Wkv[:, INNER:].reshape(8, 128, INNER).transpose(1, 0, 2).reshape(128, 8 * INNER)
    wout_r = Wout.reshape(4, 128, D).transpose(1, 0, 2).reshape(128, 4 * D)
    # keep only q,k columns: Wspec[:, :512] -> [128, 2, 512]
    wspec_r = np.ascontiguousarray(
        Wspec[:, :512].reshape(2, 128, 512).transpose(1, 0, 2).reshape(128, 1024))
    bout_r = np.ascontiguousarray(bout.reshape(8, 128).T)

    in_maps = []
    for c in range(N_CORES):
        xs = x[c * PER:(c + 1) * PER]                       # [8, 256, 1024]
        x_r = np.ascontiguousarray(
            xs.reshape(PER, 2, 128, D).transpose(0, 2, 1, 3).reshape(PER, 128, 2 * D))
        xt = xs.transpose(0, 2, 1)                          # [8, 1024, 256]
        xt_r = np.ascontiguousarray(
            xt.reshape(PER, 8, 128, N).transpose(0, 2, 1, 3).reshape(PER, 128, 8 * N))
        xc = xs[:, N // 2, :]                               # [8, 1024]
        xct_r = np.ascontiguousarray(
            xc.T.reshape(8, 128, PER).transpose(1, 0, 2).reshape(128, 8 * PER))
        in_maps.append({
            "x": x_r, "xt": xt_r, "xct": xct_r,
            "wq": wq_r, "wkt": wkt_r, "wv": wv_r, "wout": wout_r,
            "wspec": wspec_r, "bout": bout_r,
        })
    return in_maps


def kernel(x, Wq, Wkv, Wout, bout, Wspec):
    if "nc" not in _CACHE:
        _CACHE["nc"] = _build()
    nc = _CACHE["nc"]
    in_maps = _prep_inputs(x, Wq, Wkv, Wout, bout, Wspec)
    res = run_bass_kernel_spmd(nc, in_maps, list(range(N_CORES)))
    rows = np.concatenate([res.results[c]["out"] for c in range(N_CORES)], axis=0)
    # output rows are identical across the n (patch) axis
    return np.broadcast_to(rows[:, None, :], (B, N, D)).copy()


if __name__ == "__main__":
    rng = np.random.default_rng(0)
    ins = {
        "x": rng.standard_normal((B, N, D), dtype=np.float32),
        "Wq": rng.standard_normal((D, INNER), dtype=np.float32) / 32,
        "Wkv": rng.standard_normal((D, 2 * INNER), dtype=np.float32) / 32,
        "Wout": rng.standard_normal((INNER, D), dtype=np.float32) / 22.6,
        "bout": rng.standard_normal((D,), dtype=np.float32) * 0.01,
        "Wspec": rng.standard_normal((N, 3 * N), dtype=np.float32) / 16,
    }
    out = kernel(**ins)
    print("kernel output", out.shape, out.dtype)

